# revision 11
# baseline (speedup 1.0000x reference)
"""GSA (global self-attention / linear attention) Bass kernel for TRN2.

Problem: img[8,256,128,128] -> qkv 1x1-conv -> softmax(k, axis=tokens) ->
context = k_sm @ v^T (per head, 64x64) -> content = ctx^T @ q -> out 1x1-conv.

Strategy (per core, pure data-parallel over batch; 8 batches -> 8 cores):
  Pass A: stream 128-token tiles, compute k^T/v^T token-major (img block as
          matmul lhsT), exp(k) on ScalarE, accumulate ctx^T[e,d] and row-sums
          S[d] in PSUM across all 16384 tokens (ones-vector matmul for S).
  Fold:   W_eff^T[d,o] = blockdiag(ctx)/S @ w_out^T  (tiny) - this eliminates
          the whole "content" intermediate tensor.
  Pass B: recompute q ch-major from SBUF-resident img, out = W_eff^T.T @ q + b.

dtypes: projections in float32r (full PE rate, ~1.5e-4 rel err), context
matmuls in fp16 (full rate at small N, ~3e-4).
"""
import numpy as np

HEADS, DK = 8, 64
B, C, X, Y = 8, 256, 128, 128
N_TOK = X * Y          # 16384
DH = HEADS * DK        # 512
N_CORES = 8

TA = 128               # pass A token tile (fixed: partition dim of k^T/v^T)
TB = 512               # pass B token tile


def _build_program(n_tok=N_TOK, tb=TB, debug=False):
    from contextlib import ExitStack
    import concourse.bacc as bacc
    import concourse.mybir as mybir
    import concourse.tile as tile

    F32 = mybir.dt.float32
    F32R = mybir.dt.float32r
    F16 = mybir.dt.float16
    AF = mybir.ActivationFunctionType

    nta = n_tok // TA
    ntb = n_tok // tb

    nc = bacc.Bacc("TRN2", debug=False, num_devices=N_CORES)
    img_d = nc.dram_tensor("img", [C, n_tok], F32R, kind="ExternalInput").ap() \
        .rearrange("(c2 p) n -> p c2 n", p=128)
    wkv_d = nc.dram_tensor("w_kvT", [C, 2 * DH], F32R, kind="ExternalInput").ap() \
        .rearrange("(c2 p) ch -> p c2 ch", p=128)
    wq_d = nc.dram_tensor("w_qT", [C, DH], F32R, kind="ExternalInput").ap() \
        .rearrange("(c2 p) ch -> p c2 ch", p=128)
    wo_d = nc.dram_tensor("w_outT", [DH, C], F32R, kind="ExternalInput").ap() \
        .rearrange("(e4 p) o -> p e4 o", p=128)
    b_d = nc.dram_tensor("b_out", [C], F32, kind="ExternalInput").ap() \
        .rearrange("(o2 p) -> p o2", p=128)
    out_d = nc.dram_tensor("out", [C, n_tok], F32, kind="ExternalOutput").ap() \
        .rearrange("(o2 p) n -> p o2 n", p=128)
    s_scratch = nc.dram_tensor("s_scratch", [DH], F32).ap()
    dbg = {}
    if debug:
        for name, shape in [("d_ek0", [128, DH]), ("d_vt0", [128, DH]),
                            ("d_ctx", [128, 512]), ("d_rs", [128, 4]),
                            ("d_weff", [128, 4, C]), ("d_q0", [128, 4, tb])]:
            dbg[name] = nc.dram_tensor(name, shape, F32, kind="ExternalOutput").ap()

    with tile.TileContext(nc) as tc:
        with ExitStack() as ctx:
            persist = ctx.enter_context(tc.tile_pool(name="persist", bufs=1))
            small = ctx.enter_context(tc.tile_pool(name="small", bufs=1))
            acc_ctx = ctx.enter_context(ExitStack())
            psacc = acc_ctx.enter_context(tc.tile_pool(name="psacc", bufs=1, space="PSUM"))

            img_sb = persist.tile([128, 2, n_tok], F32R)
            wkv_sb = persist.tile([128, 2, 2 * DH], F32R)
            wq_sb = persist.tile([128, 2, DH], F32R)
            wo_sb = persist.tile([128, 4, C], F32R)
            b_sb = persist.tile([128, 2], F32)
            weff_sb = persist.tile([128, 4, C], F32R)
            ones_sb = persist.tile([128, 1], F16)
            zero_sb = persist.tile([128, 512], F16)

            nc.sync.dma_start(out=wkv_sb, in_=wkv_d)
            nc.sync.dma_start(out=wq_sb, in_=wq_d)
            nc.sync.dma_start(out=wo_sb, in_=wo_d)
            nc.sync.dma_start(out=b_sb, in_=b_d)
            nc.vector.memset(ones_sb, 1.0)
            nc.vector.memset(zero_sb, 0.0)

            # img load in chunks so pass A can start early
            n_chunk = 512
            for j in range(n_tok // n_chunk):
                sl = slice(j * n_chunk, (j + 1) * n_chunk)
                nc.sync.dma_start(out=img_sb[:, :, sl], in_=img_d[:, :, sl])

            # ctx^T accumulator: rows = e-pack (2 heads), cols hp*128.. = d-pack
            ctx_ps = psacc.tile([128, 4 * 128], F32)
            s_ps = psacc.tile([1, DH], F32)

            # ---------------- PASS A ----------------
            with ExitStack() as actx:
                pa = actx.enter_context(tc.tile_pool(name="pa", bufs=3))
                psa = actx.enter_context(tc.tile_pool(name="psa", bufs=2, space="PSUM"))
                # one accumulation group per PSUM bank: a zero-valued matmul
                # opens the ctx bank (start) and another closes it (stop);
                # every real ctx matmul purely accumulates. WAW deps on the
                # full-bank APs enforce ordering.
                nc.tensor.matmul(ctx_ps, lhsT=zero_sb[:, 0:128], rhs=zero_sb,
                                 start=True, stop=False, skip_group_check=True)
                for i in range(nta):
                    sl = slice(i * TA, (i + 1) * TA)
                    kv_ps = psa.tile([128, 2 * DH], F32)
                    for c2 in range(2):
                        nc.tensor.matmul(kv_ps[:, 0:DH],
                                         lhsT=img_sb[:, c2, sl],
                                         rhs=wkv_sb[:, c2, 0:DH],
                                         start=(c2 == 0), stop=(c2 == 1))
                    for c2 in range(2):
                        nc.tensor.matmul(kv_ps[:, DH:2 * DH],
                                         lhsT=img_sb[:, c2, sl],
                                         rhs=wkv_sb[:, c2, DH:2 * DH],
                                         start=(c2 == 0), stop=(c2 == 1))
                    ek = pa.tile([128, DH], F16, tag="ek")
                    nc.scalar.activation(out=ek, in_=kv_ps[:, 0:DH], func=AF.Exp)
                    vt = pa.tile([128, DH], F16, tag="vt")
                    nc.vector.tensor_copy(out=vt, in_=kv_ps[:, DH:2 * DH])
                    if debug and i == 0:
                        ek32 = pa.tile([128, DH], F32, tag="ek32")
                        vt32 = pa.tile([128, DH], F32, tag="vt32")
                        nc.vector.tensor_copy(out=ek32, in_=ek)
                        nc.vector.tensor_copy(out=vt32, in_=vt)
                        nc.sync.dma_start(out=dbg["d_ek0"], in_=ek32)
                        nc.sync.dma_start(out=dbg["d_vt0"], in_=vt32)
                    for hp in range(4):
                        hsl = slice(hp * 128, (hp + 1) * 128)
                        nc.tensor.matmul(ctx_ps[:, hsl],
                                         lhsT=vt[:, hsl], rhs=ek[:, hsl],
                                         start=False, stop=False,
                                         skip_group_check=True)
                    nc.tensor.matmul(s_ps, lhsT=ones_sb, rhs=ek,
                                     start=(i == 0), stop=(i == nta - 1),
                                     skip_group_check=True)
                if True:
                    nc.tensor.matmul(ctx_ps, lhsT=zero_sb[:, 0:128], rhs=zero_sb,
                                     start=False, stop=True, skip_group_check=True)

            # ---------------- FOLD: W_eff^T = (blockdiag(ctx)/S) @ w_out^T --------
            with ExitStack() as wctx:
                psw = wctx.enter_context(tc.tile_pool(name="psw", bufs=1, space="PSUM"))
                ctx_sb = small.tile([128, 4 * 128], F32R)
                nc.vector.memset(ctx_sb.bitcast(F32), 0.0)
                for hp in range(4):
                    for half in range(2):
                        psl = slice(half * 64, (half + 1) * 64)
                        csl = slice(hp * 128 + half * 64, hp * 128 + (half + 1) * 64)
                        nc.vector.tensor_copy(out=ctx_sb[psl, csl], in_=ctx_ps[psl, csl])
                s_sb = small.tile([1, DH], F32)
                nc.vector.tensor_copy(out=s_sb, in_=s_ps)
                nc.sync.dma_start(out=s_scratch, in_=s_sb)
                scol = small.tile([128, 4], F32)
                nc.sync.dma_start(out=scol, in_=s_scratch.rearrange("(f p) -> p f", p=128))
                rs = small.tile([128, 4], F32)
                nc.vector.reciprocal(out=rs, in_=scol)
                if debug:
                    ctx32 = small.tile([128, 512], F32)
                    nc.vector.tensor_copy(out=ctx32, in_=ctx_sb)
                    nc.sync.dma_start(out=dbg["d_ctx"], in_=ctx32)
                    nc.sync.dma_start(out=dbg["d_rs"], in_=rs)

                psw_t = psw.tile([128, 4, 512], F32)
                for hp in range(4):
                    hsl = slice(hp * 128, (hp + 1) * 128)
                    nc.tensor.matmul(psw_t[:, hp, 0:C],
                                     lhsT=ctx_sb[:, hsl], rhs=wo_sb[:, hp, :],
                                     start=True, stop=True)
                for hp in range(4):
                    nc.vector.tensor_scalar_mul(out=weff_sb[:, hp, :],
                                                in0=psw_t[:, hp, 0:C],
                                                scalar1=rs[:, hp:hp + 1])

            if debug:
                weff32 = small.tile([128, 4, C], F32)
                nc.vector.tensor_copy(out=weff32, in_=weff_sb)
                nc.sync.dma_start(out=dbg["d_weff"], in_=weff32)

            acc_ctx.close()  # release ctx/S PSUM banks before pass B

            # ---------------- PASS B ----------------
            with ExitStack() as bctx:
                pb = bctx.enter_context(tc.tile_pool(name="pb", bufs=2))
                psq = bctx.enter_context(tc.tile_pool(name="psq", bufs=1, space="PSUM"))
                pso = bctx.enter_context(tc.tile_pool(name="pso", bufs=2, space="PSUM"))
                for i in range(ntb):
                    sl = slice(i * tb, (i + 1) * tb)
                    q_ps = psq.tile([128, 4, tb], F32)
                    for m4 in range(4):
                        msl = slice(m4 * 128, (m4 + 1) * 128)
                        for c2 in range(2):
                            nc.tensor.matmul(q_ps[:, m4, :],
                                             lhsT=wq_sb[:, c2, msl],
                                             rhs=img_sb[:, c2, sl],
                                             start=(c2 == 0), stop=(c2 == 1))
                    q_sb = pb.tile([128, 4, tb], F32R, tag="q")
                    nc.vector.tensor_copy(out=q_sb[:, 0:2, :], in_=q_ps[:, 0:2, :])
                    nc.scalar.activation(out=q_sb[:, 2:4, :], in_=q_ps[:, 2:4, :],
                                         func=AF.Identity)
                    if debug and i == 0:
                        nc.sync.dma_start(out=dbg["d_q0"], in_=q_sb.bitcast(F32))
                    out_ps = pso.tile([128, 2, tb], F32)
                    for o2 in range(2):
                        osl = slice(o2 * 128, (o2 + 1) * 128)
                        for d4 in range(4):
                            nc.tensor.matmul(out_ps[:, o2, :],
                                             lhsT=weff_sb[:, d4, osl],
                                             rhs=q_sb[:, d4, :],
                                             start=(d4 == 0), stop=(d4 == 3))
                    out_sb = pb.tile([128, 2, tb], F32, tag="o")
                    for o2 in range(2):
                        nc.scalar.activation(out=out_sb[:, o2, :], in_=out_ps[:, o2, :],
                                             func=AF.Identity,
                                             bias=b_sb[:, o2:o2 + 1])
                    nc.sync.dma_start(out=out_d[:, :, sl], in_=out_sb)

    nc.compile()
    return nc


def _prep_inputs(img, w_qkv, w_out, b_out, n_tok=N_TOK):
    imgs = np.ascontiguousarray(img.reshape(B, C, n_tok), dtype=np.float32)
    w_qkv = np.asarray(w_qkv, dtype=np.float32)
    w_kvT = np.ascontiguousarray(w_qkv[DH:3 * DH].T)      # [256, 1024]
    w_qT = np.ascontiguousarray(w_qkv[0:DH].T)            # [256, 512]
    w_outT = np.ascontiguousarray(np.asarray(w_out, dtype=np.float32).T)  # [512, 256]
    b = np.ascontiguousarray(np.asarray(b_out, dtype=np.float32))
    return [
        {"img": imgs[i], "w_kvT": w_kvT, "w_qT": w_qT, "w_outT": w_outT, "b_out": b}
        for i in range(N_CORES)
    ]


class _Exec:
    """Compile once, execute many times on the 8 cores via PJRT/shard_map."""

    def __init__(self, nc):
        import jax
        import concourse.mybir as mybir
        from jax.experimental.shard_map import shard_map
        from jax.sharding import Mesh, PartitionSpec, NamedSharding
        from concourse.bass2jax import _bass_exec_p, install_neuronx_cc_hook, partition_id_tensor

        install_neuronx_cc_hook()
        self.jax = jax
        in_names, out_names, out_avals = [], [], []
        partition_name = nc.partition_id_tensor.name if nc.partition_id_tensor else None
        for alloc in nc.m.functions[0].allocations:
            if not isinstance(alloc, mybir.MemoryLocationSet):
                continue
            name = alloc.memorylocations[0].name
            if alloc.kind == "ExternalInput":
                if name != partition_name:
                    in_names.append(name)
            elif alloc.kind == "ExternalOutput":
                out_names.append(name)
                out_avals.append(jax.core.ShapedArray(
                    tuple(alloc.tensor_shape), mybir.dt.np(alloc.dtype)))
        self.in_names, self.out_names, self.out_avals = in_names, out_names, out_avals
        n_params = len(in_names)
        all_in_names = in_names + out_names
        if partition_name is not None:
            all_in_names.append(partition_name)

        def _body(*args):
            operands = list(args)
            if partition_name is not None:
                operands.append(partition_id_tensor())
            return tuple(_bass_exec_p.bind(
                *operands,
                out_avals=tuple(out_avals),
                in_names=tuple(all_in_names),
                out_names=tuple(out_names),
                lowering_input_output_aliases=(),
                sim_require_finite=True,
                sim_require_nnan=True,
                nc=nc,
            ))

        devices = jax.devices()[:N_CORES]
        mesh = Mesh(np.asarray(devices), ("core",))
        self.sharding = NamedSharding(mesh, PartitionSpec("core"))
        n_ops = n_params + len(out_names)
        self.fn = jax.jit(
            shard_map(_body, mesh=mesh,
                      in_specs=(PartitionSpec("core"),) * n_ops,
                      out_specs=(PartitionSpec("core"),) * len(out_names),
                      check_rep=False),
            keep_unused=True,
        )
        self.dev_zeros = [
            jax.device_put(np.zeros((N_CORES * a.shape[0], *a.shape[1:]), a.dtype),
                           self.sharding)
            for a in out_avals
        ]

    def stage(self, in_maps):
        concat = [
            np.concatenate([np.asarray(m[name]) for m in in_maps], axis=0)
            for name in self.in_names
        ]
        return [self.jax.device_put(a, self.sharding) for a in concat]

    def run(self, staged):
        outs = self.fn(*staged, *self.dev_zeros)
        self.jax.block_until_ready(outs)
        return outs

    def results(self, outs):
        per_core = []
        for c in range(N_CORES):
            per_core.append({
                name: np.asarray(outs[i]).reshape(N_CORES, *self.out_avals[i].shape)[c]
                for i, name in enumerate(self.out_names)
            })
        return per_core


_CACHE = {}


def _get_exec():
    if "exec" not in _CACHE:
        _CACHE["exec"] = _Exec(_build_program())
    return _CACHE["exec"]


def kernel(img, w_qkv, w_out, b_out):
    ex = _get_exec()
    staged = ex.stage(_prep_inputs(img, w_qkv, w_out, b_out))
    res = ex.results(ex.run(staged))
    out = np.stack([res[i]["out"] for i in range(N_CORES)])
    return out.reshape(B, C, X, Y)


# revision 17
# speedup vs baseline: 98.3839x; 98.3839x over previous
"""GSA (global self-attention / linear attention) Bass kernel for TRN2.

Problem: img[8,256,128,128] -> qkv 1x1-conv -> softmax(k, axis=tokens) ->
context = k_sm @ v^T (per head, 64x64) -> content = ctx^T @ q -> out 1x1-conv.

Strategy (per core, pure data-parallel over batch; 8 batches -> 8 cores):
  Pass A: stream 128-token tiles; k^T/v^T token-major (img block is the matmul
          lhsT), exp(k) on ScalarE; accumulate ctx^T[e,d] (2-head packs) and
          row-sums S[d] in PSUM across all 16384 tokens (ones-vector matmul).
  Fold:   everything after the softmax is linear in img, so collapse it:
          W_eff^T = (blockdiag(ctx)/S) @ w_out^T, then
          W_comb^T[c,o] = sum_d w_q[d,c] W_eff^T[d,o]  (256x256, tiny).
  Pass B: out = W_comb @ img + b  - a single fused projection, no q, no
          content tensor.

dtypes: projections float32r (full PE rate at N>=256, ~1.5e-4), context
matmuls fp16 (full rate at N=128, ~3e-4 and exact-range-safe).
"""
import numpy as np

HEADS, DK = 8, 64
B, C, X, Y = 8, 256, 128, 128
N_TOK = X * Y          # 16384
DH = HEADS * DK        # 512
N_CORES = 8

TA = 128               # pass A token tile (fixed: partition dim of k^T/v^T)
TB = 512               # pass B token tile


def _build_program(n_tok=N_TOK, tb=TB, debug=False, pa_bufs=6, psa_bufs=3,
                   pb_bufs=4, pso_bufs=3, phases="ab", repeat=1,
                   img_chunk=512):
    from contextlib import ExitStack
    import concourse.bacc as bacc
    import concourse.mybir as mybir
    import concourse.tile as tile

    F32 = mybir.dt.float32
    F32R = mybir.dt.float32r
    F16 = mybir.dt.float16
    AF = mybir.ActivationFunctionType

    nta = n_tok // TA if "a" in phases else 1
    ntb = n_tok // tb if "b" in phases else 0

    nc = bacc.Bacc("TRN2", debug=False, num_devices=N_CORES)
    img_d = nc.dram_tensor("img", [C, n_tok], F32R, kind="ExternalInput").ap() \
        .rearrange("(c2 p) n -> p c2 n", p=128)
    wkv_d = nc.dram_tensor("w_kvT", [C, 2 * DH], F32R, kind="ExternalInput").ap() \
        .rearrange("(c2 p) ch -> p c2 ch", p=128)
    wq_d = nc.dram_tensor("w_q", [DH, C], F32R, kind="ExternalInput").ap() \
        .rearrange("(d4 p) c -> p d4 c", p=128)
    wo_d = nc.dram_tensor("w_outT", [DH, C], F32R, kind="ExternalInput").ap() \
        .rearrange("(e4 p) o -> p e4 o", p=128)
    b_d = nc.dram_tensor("b_out", [C], F32, kind="ExternalInput").ap() \
        .rearrange("(o2 p) -> p o2", p=128)
    out_d = nc.dram_tensor("out", [C, n_tok], F32, kind="ExternalOutput").ap() \
        .rearrange("(o2 p) n -> p o2 n", p=128)
    s_scratch = nc.dram_tensor("s_scratch", [DH], F32).ap()
    dbg = {}
    if debug:
        for name, shape in [("d_ek0", [128, DH]), ("d_vt0", [128, DH]),
                            ("d_ctx", [128, 512]), ("d_rs", [128, 4]),
                            ("d_weff", [128, 4, C])]:
            dbg[name] = nc.dram_tensor(name, shape, F32, kind="ExternalOutput").ap()

    def emit(tc, ctx):
        persist = ctx.enter_context(tc.tile_pool(name="persist", bufs=1))
        small = ctx.enter_context(tc.tile_pool(name="small", bufs=1))
        acc_ctx = ctx.enter_context(ExitStack())
        psacc = acc_ctx.enter_context(tc.tile_pool(name="psacc", bufs=1, space="PSUM"))

        img_sb = persist.tile([128, 2, n_tok], F32R)
        wkv_sb = persist.tile([128, 2, 2 * DH], F32R)
        wq_sb = persist.tile([128, 4, C], F32R)
        wcombT_sb = persist.tile([128, 2, C], F32R)
        wo_sb = persist.tile([128, 4, C], F32R)
        b_sb = persist.tile([128, 2], F32)
        weff_sb = persist.tile([128, 4, C], F32R)
        ones_sb = persist.tile([128, 1], F16)
        zero_sb = persist.tile([128, 512], F16)

        nc.sync.dma_start(out=wkv_sb, in_=wkv_d)
        nc.sync.dma_start(out=wq_sb, in_=wq_d)
        nc.sync.dma_start(out=wo_sb, in_=wo_d)
        nc.sync.dma_start(out=b_sb, in_=b_d)
        nc.vector.memset(ones_sb, 1.0)
        nc.vector.memset(zero_sb, 0.0)

        # img load in chunks so pass A can start early
        for j in range(n_tok // img_chunk):
            sl = slice(j * img_chunk, (j + 1) * img_chunk)
            nc.sync.dma_start(out=img_sb[:, :, sl], in_=img_d[:, :, sl])

        # ctx^T accumulator: rows = e-pack (2 heads), cols hp*128.. = d-pack
        ctx_ps = psacc.tile([128, 4 * 128], F32)
        s_ps = psacc.tile([1, DH], F32)

        # ---------------- PASS A ----------------
        with ExitStack() as actx:
            pa = actx.enter_context(tc.tile_pool(name="pa", bufs=pa_bufs))
            psa = actx.enter_context(tc.tile_pool(name="psa", bufs=psa_bufs, space="PSUM"))
            # One accumulation group per PSUM bank: a zero-valued matmul opens
            # the ctx bank (start) and another closes it (stop); every real
            # ctx matmul purely accumulates. WAW deps on the full-bank APs
            # enforce ordering.
            nc.tensor.matmul(ctx_ps, lhsT=zero_sb[:, 0:128], rhs=zero_sb,
                             start=True, stop=False, skip_group_check=True)
            for i in range(nta):
                sl = slice(i * TA, (i + 1) * TA)
                k_ps = psa.tile([128, DH], F32, tag="kps")
                v_ps = psa.tile([128, DH], F32, tag="vps")
                for c2 in range(2):
                    nc.tensor.matmul(k_ps, lhsT=img_sb[:, c2, sl],
                                     rhs=wkv_sb[:, c2, 0:DH],
                                     start=(c2 == 0), stop=(c2 == 1))
                for c2 in range(2):
                    nc.tensor.matmul(v_ps, lhsT=img_sb[:, c2, sl],
                                     rhs=wkv_sb[:, c2, DH:2 * DH],
                                     start=(c2 == 0), stop=(c2 == 1))
                ek = pa.tile([128, DH], F16, tag="ek")
                nc.scalar.activation(out=ek, in_=k_ps, func=AF.Exp)
                vt = pa.tile([128, DH], F16, tag="vt")
                nc.vector.tensor_copy(out=vt[:, 0:256], in_=v_ps[:, 0:256])
                nc.scalar.activation(out=vt[:, 256:512], in_=v_ps[:, 256:512],
                                     func=AF.Identity)
                if debug and i == 0:
                    ek32 = pa.tile([128, DH], F32, tag="ek32")
                    vt32 = pa.tile([128, DH], F32, tag="vt32")
                    nc.vector.tensor_copy(out=ek32, in_=ek)
                    nc.vector.tensor_copy(out=vt32, in_=vt)
                    nc.sync.dma_start(out=dbg["d_ek0"], in_=ek32)
                    nc.sync.dma_start(out=dbg["d_vt0"], in_=vt32)
                for hp in range(4):
                    hsl = slice(hp * 128, (hp + 1) * 128)
                    nc.tensor.matmul(ctx_ps[:, hsl],
                                     lhsT=vt[:, hsl], rhs=ek[:, hsl],
                                     start=False, stop=False,
                                     skip_group_check=True)
                nc.tensor.matmul(s_ps, lhsT=ones_sb, rhs=ek,
                                 start=(i == 0), stop=(i == nta - 1),
                                 skip_group_check=True)
            nc.tensor.matmul(ctx_ps, lhsT=zero_sb[:, 0:128], rhs=zero_sb,
                             start=False, stop=True, skip_group_check=True)

        # ---- FOLD: W_eff^T = (blockdiag(ctx)/S) @ w_out^T; then W_comb^T
        with ExitStack() as wctx:
            psw = wctx.enter_context(tc.tile_pool(name="psw", bufs=1, space="PSUM"))
            ctx_sb = small.tile([128, 4 * 128], F32R)
            nc.vector.memset(ctx_sb.bitcast(F32), 0.0)
            for hp in range(4):
                for half in range(2):
                    psl = slice(half * 64, (half + 1) * 64)
                    csl = slice(hp * 128 + half * 64, hp * 128 + (half + 1) * 64)
                    nc.vector.tensor_copy(out=ctx_sb[psl, csl], in_=ctx_ps[psl, csl])
            s_sb = small.tile([1, DH], F32)
            nc.vector.tensor_copy(out=s_sb, in_=s_ps)
            nc.sync.dma_start(out=s_scratch, in_=s_sb)
            scol = small.tile([128, 4], F32)
            nc.sync.dma_start(out=scol, in_=s_scratch.rearrange("(f p) -> p f", p=128))
            rs = small.tile([128, 4], F32)
            nc.vector.reciprocal(out=rs, in_=scol)
            if debug:
                ctx32 = small.tile([128, 512], F32)
                nc.vector.tensor_copy(out=ctx32, in_=ctx_sb)
                nc.sync.dma_start(out=dbg["d_ctx"], in_=ctx32)
                nc.sync.dma_start(out=dbg["d_rs"], in_=rs)

            psw_t = psw.tile([128, 4, 512], F32)
            for hp in range(4):
                hsl = slice(hp * 128, (hp + 1) * 128)
                nc.tensor.matmul(psw_t[:, hp, 0:C],
                                 lhsT=ctx_sb[:, hsl], rhs=wo_sb[:, hp, :],
                                 start=True, stop=True)
            for hp in range(4):
                nc.vector.tensor_scalar_mul(out=weff_sb[:, hp, :],
                                            in0=psw_t[:, hp, 0:C],
                                            scalar1=rs[:, hp:hp + 1])

            # W_comb^T[c,o] = sum_d w_q[d,c] * W_eff^T[d,o]
            wc_ps = psw.tile([128, 2, 512], F32)
            for c2 in range(2):
                csl = slice(c2 * 128, (c2 + 1) * 128)
                for d4 in range(4):
                    nc.tensor.matmul(wc_ps[:, c2, 0:C],
                                     lhsT=wq_sb[:, d4, csl],
                                     rhs=weff_sb[:, d4, :],
                                     start=(d4 == 0), stop=(d4 == 3))
            for c2 in range(2):
                nc.vector.tensor_copy(out=wcombT_sb[:, c2, :], in_=wc_ps[:, c2, 0:C])

        if debug:
            weff32 = small.tile([128, 4, C], F32)
            nc.vector.tensor_copy(out=weff32, in_=weff_sb)
            nc.sync.dma_start(out=dbg["d_weff"], in_=weff32)

        acc_ctx.close()  # release ctx/S PSUM banks before pass B

        # ---------------- PASS B: out = W_comb @ img + b ----------------
        with ExitStack() as bctx:
            pb = bctx.enter_context(tc.tile_pool(name="pb", bufs=pb_bufs))
            pso = bctx.enter_context(tc.tile_pool(name="pso", bufs=pso_bufs, space="PSUM"))
            for i in range(ntb):
                sl = slice(i * tb, (i + 1) * tb)
                out_ps = pso.tile([128, 2, tb], F32)
                for o2 in range(2):
                    osl = slice(o2 * 128, (o2 + 1) * 128)
                    for c2 in range(2):
                        nc.tensor.matmul(out_ps[:, o2, :],
                                         lhsT=wcombT_sb[:, c2, osl],
                                         rhs=img_sb[:, c2, sl],
                                         start=(c2 == 0), stop=(c2 == 1))
                out_sb = pb.tile([128, 2, tb], F32, tag="o")
                for o2 in range(2):
                    nc.scalar.activation(out=out_sb[:, o2, :], in_=out_ps[:, o2, :],
                                         func=AF.Identity,
                                         bias=b_sb[:, o2:o2 + 1])
                nc.sync.dma_start(out=out_d[:, :, sl], in_=out_sb)

    with tile.TileContext(nc) as tc:
        for _rep in range(repeat):
            with ExitStack() as ctx:
                emit(tc, ctx)

    nc.compile()
    return nc


def _prep_inputs(img, w_qkv, w_out, b_out, n_tok=N_TOK):
    imgs = np.ascontiguousarray(img.reshape(B, C, n_tok), dtype=np.float32)
    w_qkv = np.asarray(w_qkv, dtype=np.float32)
    w_kvT = np.ascontiguousarray(w_qkv[DH:3 * DH].T)      # [256, 1024]
    w_q = np.ascontiguousarray(w_qkv[0:DH])               # [512, 256]
    w_outT = np.ascontiguousarray(np.asarray(w_out, dtype=np.float32).T)  # [512, 256]
    b = np.ascontiguousarray(np.asarray(b_out, dtype=np.float32))
    return [
        {"img": imgs[i], "w_kvT": w_kvT, "w_q": w_q, "w_outT": w_outT, "b_out": b}
        for i in range(N_CORES)
    ]


class _Exec:
    """Compile once, execute many times on the 8 cores via PJRT/shard_map."""

    def __init__(self, nc):
        import jax
        import concourse.mybir as mybir
        from jax.experimental.shard_map import shard_map
        from jax.sharding import Mesh, PartitionSpec, NamedSharding
        from concourse.bass2jax import _bass_exec_p, install_neuronx_cc_hook, partition_id_tensor

        install_neuronx_cc_hook()
        self.jax = jax
        in_names, out_names, out_avals = [], [], []
        partition_name = nc.partition_id_tensor.name if nc.partition_id_tensor else None
        for alloc in nc.m.functions[0].allocations:
            if not isinstance(alloc, mybir.MemoryLocationSet):
                continue
            name = alloc.memorylocations[0].name
            if alloc.kind == "ExternalInput":
                if name != partition_name:
                    in_names.append(name)
            elif alloc.kind == "ExternalOutput":
                out_names.append(name)
                out_avals.append(jax.core.ShapedArray(
                    tuple(alloc.tensor_shape), mybir.dt.np(alloc.dtype)))
        self.in_names, self.out_names, self.out_avals = in_names, out_names, out_avals
        n_params = len(in_names)
        all_in_names = in_names + out_names
        if partition_name is not None:
            all_in_names.append(partition_name)

        def _body(*args):
            operands = list(args)
            if partition_name is not None:
                operands.append(partition_id_tensor())
            return tuple(_bass_exec_p.bind(
                *operands,
                out_avals=tuple(out_avals),
                in_names=tuple(all_in_names),
                out_names=tuple(out_names),
                lowering_input_output_aliases=(),
                sim_require_finite=True,
                sim_require_nnan=True,
                nc=nc,
            ))

        devices = jax.devices()[:N_CORES]
        mesh = Mesh(np.asarray(devices), ("core",))
        self._body = _body
        self.mesh = mesh
        self.sharding = NamedSharding(mesh, PartitionSpec("core"))
        n_ops = n_params + len(out_names)
        self.fn = jax.jit(
            shard_map(_body, mesh=mesh,
                      in_specs=(PartitionSpec("core"),) * n_ops,
                      out_specs=(PartitionSpec("core"),) * len(out_names),
                      check_rep=False),
            keep_unused=True,
        )
        self.dev_zeros = [
            jax.device_put(np.zeros((N_CORES * a.shape[0], *a.shape[1:]), a.dtype),
                           self.sharding)
            for a in out_avals
        ]

    def stage(self, in_maps):
        concat = [
            np.concatenate([np.asarray(m[name]) for m in in_maps], axis=0)
            for name in self.in_names
        ]
        return [self.jax.device_put(a, self.sharding) for a in concat]

    def run(self, staged):
        outs = self.fn(*staged, *self.dev_zeros)
        self.jax.block_until_ready(outs)
        return outs

    def results(self, outs):
        per_core = []
        for c in range(N_CORES):
            per_core.append({
                name: np.asarray(outs[i]).reshape(N_CORES, *self.out_avals[i].shape)[c]
                for i, name in enumerate(self.out_names)
            })
        return per_core


_CACHE = {}


def _get_exec():
    if "exec" not in _CACHE:
        _CACHE["exec"] = _Exec(_build_program())
    return _CACHE["exec"]


def kernel(img, w_qkv, w_out, b_out):
    ex = _get_exec()
    staged = ex.stage(_prep_inputs(img, w_qkv, w_out, b_out))
    res = ex.results(ex.run(staged))
    out = np.stack([res[i]["out"] for i in range(N_CORES)])
    return out.reshape(B, C, X, Y)


# revision 18
# speedup vs baseline: 224.1213x; 2.2780x over previous
"""GSA (global self-attention / linear attention) Bass kernel for TRN2.

Problem: img[8,256,128,128] -> qkv 1x1-conv -> softmax(k, axis=tokens) ->
context = k_sm @ v^T (per head, 64x64) -> content = ctx^T @ q -> out 1x1-conv.

Strategy (per core, pure data-parallel over batch; 8 batches -> 8 cores):
  Pass A: stream 128-token tiles; k^T/v^T token-major (img block is the matmul
          lhsT), exp(k) on ScalarE; accumulate ctx^T[e,d] (2-head packs) and
          row-sums S[d] in PSUM across all 16384 tokens (ones-vector matmul).
  Fold:   everything after the softmax is linear in img, so collapse it:
          W_eff^T = (blockdiag(ctx)/S) @ w_out^T, then
          W_comb^T[c,o] = sum_d w_q[d,c] W_eff^T[d,o]  (256x256, tiny).
  Pass B: out = W_comb @ img + b  - a single fused projection, no q, no
          content tensor.

dtypes: projections float32r (full PE rate at N>=256, ~1.5e-4), context
matmuls fp16 (full rate at N=128, ~3e-4 and exact-range-safe).
"""
import numpy as np

HEADS, DK = 8, 64
B, C, X, Y = 8, 256, 128, 128
N_TOK = X * Y          # 16384
DH = HEADS * DK        # 512
N_CORES = 8

TA = 128               # pass A token tile (fixed: partition dim of k^T/v^T)
TB = 512               # pass B token tile


def _build_program(n_tok=N_TOK, tb=TB, debug=False, pa_bufs=6, psa_bufs=3,
                   pb_bufs=4, pso_bufs=3, phases="ab", repeat=1,
                   img_chunk=512, io="ext"):
    from contextlib import ExitStack
    import concourse.bacc as bacc
    import concourse.mybir as mybir
    import concourse.tile as tile

    F32 = mybir.dt.float32
    F32R = mybir.dt.float32r
    F16 = mybir.dt.float16
    AF = mybir.ActivationFunctionType

    nta = n_tok // TA if "a" in phases else 1
    ntb = n_tok // tb if "b" in phases else 0

    nc = bacc.Bacc("TRN2", debug=False, num_devices=N_CORES)
    io_kind = dict(kind="ExternalInput") if io == "ext" else {}
    io_okind = dict(kind="ExternalOutput") if io == "ext" else {}
    img_d = nc.dram_tensor("img", [C, n_tok], F32R, **io_kind).ap() \
        .rearrange("(c2 p) n -> p c2 n", p=128)
    wkv_d = nc.dram_tensor("w_kvT", [C, 2 * DH], F32R, kind="ExternalInput").ap() \
        .rearrange("(c2 p) ch -> p c2 ch", p=128)
    wq_d = nc.dram_tensor("w_q", [DH, C], F32R, kind="ExternalInput").ap() \
        .rearrange("(d4 p) c -> p d4 c", p=128)
    wo_d = nc.dram_tensor("w_outT", [DH, C], F32R, kind="ExternalInput").ap() \
        .rearrange("(e4 p) o -> p e4 o", p=128)
    b_d = nc.dram_tensor("b_out", [C], F32, kind="ExternalInput").ap() \
        .rearrange("(o2 p) -> p o2", p=128)
    out_d = nc.dram_tensor("out", [C, n_tok], F32, **io_okind).ap() \
        .rearrange("(o2 p) n -> p o2 n", p=128)
    s_scratch = nc.dram_tensor("s_scratch", [DH], F32).ap()
    marker_d = None
    if io != "ext":
        marker_d = nc.dram_tensor("marker", [1, 4], F32, kind="ExternalOutput").ap()
    dbg = {}
    if debug:
        for name, shape in [("d_ek0", [128, DH]), ("d_vt0", [128, DH]),
                            ("d_ctx", [128, 512]), ("d_rs", [128, 4]),
                            ("d_weff", [128, 4, C])]:
            dbg[name] = nc.dram_tensor(name, shape, F32, kind="ExternalOutput").ap()

    def emit(tc, ctx):
        persist = ctx.enter_context(tc.tile_pool(name="persist", bufs=1))
        small = ctx.enter_context(tc.tile_pool(name="small", bufs=1))
        acc_ctx = ctx.enter_context(ExitStack())
        psacc = acc_ctx.enter_context(tc.tile_pool(name="psacc", bufs=1, space="PSUM"))

        img_sb = persist.tile([128, 2, n_tok], F32R)
        wkv_sb = persist.tile([128, 2, 2 * DH], F32R)
        wq_sb = persist.tile([128, 4, C], F32R)
        wcombT_sb = persist.tile([128, 2, C], F32R)
        wo_sb = persist.tile([128, 4, C], F32R)
        b_sb = persist.tile([128, 2], F32)
        weff_sb = persist.tile([128, 4, C], F32R)
        ones_sb = persist.tile([128, 1], F16)
        zero_sb = persist.tile([128, 512], F16)

        nc.sync.dma_start(out=wkv_sb, in_=wkv_d)
        nc.sync.dma_start(out=wq_sb, in_=wq_d)
        nc.sync.dma_start(out=wo_sb, in_=wo_d)
        nc.sync.dma_start(out=b_sb, in_=b_d)
        nc.vector.memset(ones_sb, 1.0)
        nc.vector.memset(zero_sb, 0.0)

        # img load in chunks so pass A can start early
        for j in range(n_tok // img_chunk):
            sl = slice(j * img_chunk, (j + 1) * img_chunk)
            nc.sync.dma_start(out=img_sb[:, :, sl], in_=img_d[:, :, sl])

        # ctx^T accumulator: rows = e-pack (2 heads), cols hp*128.. = d-pack
        ctx_ps = psacc.tile([128, 4 * 128], F32)
        s_ps = psacc.tile([1, DH], F32)

        # ---------------- PASS A ----------------
        with ExitStack() as actx:
            pa = actx.enter_context(tc.tile_pool(name="pa", bufs=pa_bufs))
            psa = actx.enter_context(tc.tile_pool(name="psa", bufs=psa_bufs, space="PSUM"))
            # One accumulation group per PSUM bank: a zero-valued matmul opens
            # the ctx bank (start) and another closes it (stop); every real
            # ctx matmul purely accumulates. WAW deps on the full-bank APs
            # enforce ordering.
            nc.tensor.matmul(ctx_ps, lhsT=zero_sb[:, 0:128], rhs=zero_sb,
                             start=True, stop=False, skip_group_check=True)
            for i in range(nta):
                sl = slice(i * TA, (i + 1) * TA)
                k_ps = psa.tile([128, DH], F32, tag="kps")
                v_ps = psa.tile([128, DH], F32, tag="vps")
                for c2 in range(2):
                    nc.tensor.matmul(k_ps, lhsT=img_sb[:, c2, sl],
                                     rhs=wkv_sb[:, c2, 0:DH],
                                     start=(c2 == 0), stop=(c2 == 1))
                for c2 in range(2):
                    nc.tensor.matmul(v_ps, lhsT=img_sb[:, c2, sl],
                                     rhs=wkv_sb[:, c2, DH:2 * DH],
                                     start=(c2 == 0), stop=(c2 == 1))
                ek = pa.tile([128, DH], F16, tag="ek")
                nc.scalar.activation(out=ek, in_=k_ps, func=AF.Exp)
                vt = pa.tile([128, DH], F16, tag="vt")
                nc.vector.tensor_copy(out=vt[:, 0:256], in_=v_ps[:, 0:256])
                nc.scalar.activation(out=vt[:, 256:512], in_=v_ps[:, 256:512],
                                     func=AF.Identity)
                if debug and i == 0:
                    ek32 = pa.tile([128, DH], F32, tag="ek32")
                    vt32 = pa.tile([128, DH], F32, tag="vt32")
                    nc.vector.tensor_copy(out=ek32, in_=ek)
                    nc.vector.tensor_copy(out=vt32, in_=vt)
                    nc.sync.dma_start(out=dbg["d_ek0"], in_=ek32)
                    nc.sync.dma_start(out=dbg["d_vt0"], in_=vt32)
                for hp in range(4):
                    hsl = slice(hp * 128, (hp + 1) * 128)
                    nc.tensor.matmul(ctx_ps[:, hsl],
                                     lhsT=vt[:, hsl], rhs=ek[:, hsl],
                                     start=False, stop=False,
                                     skip_group_check=True)
                nc.tensor.matmul(s_ps, lhsT=ones_sb, rhs=ek,
                                 start=(i == 0), stop=(i == nta - 1),
                                 skip_group_check=True)
            nc.tensor.matmul(ctx_ps, lhsT=zero_sb[:, 0:128], rhs=zero_sb,
                             start=False, stop=True, skip_group_check=True)

        # ---- FOLD: W_eff^T = (blockdiag(ctx)/S) @ w_out^T; then W_comb^T
        with ExitStack() as wctx:
            psw = wctx.enter_context(tc.tile_pool(name="psw", bufs=1, space="PSUM"))
            ctx_sb = small.tile([128, 4 * 128], F32R)
            nc.vector.memset(ctx_sb.bitcast(F32), 0.0)
            for hp in range(4):
                for half in range(2):
                    psl = slice(half * 64, (half + 1) * 64)
                    csl = slice(hp * 128 + half * 64, hp * 128 + (half + 1) * 64)
                    nc.vector.tensor_copy(out=ctx_sb[psl, csl], in_=ctx_ps[psl, csl])
            s_sb = small.tile([1, DH], F32)
            nc.vector.tensor_copy(out=s_sb, in_=s_ps)
            nc.sync.dma_start(out=s_scratch, in_=s_sb)
            scol = small.tile([128, 4], F32)
            nc.sync.dma_start(out=scol, in_=s_scratch.rearrange("(f p) -> p f", p=128))
            rs = small.tile([128, 4], F32)
            nc.vector.reciprocal(out=rs, in_=scol)
            if debug:
                ctx32 = small.tile([128, 512], F32)
                nc.vector.tensor_copy(out=ctx32, in_=ctx_sb)
                nc.sync.dma_start(out=dbg["d_ctx"], in_=ctx32)
                nc.sync.dma_start(out=dbg["d_rs"], in_=rs)

            psw_t = psw.tile([128, 4, 512], F32)
            for hp in range(4):
                hsl = slice(hp * 128, (hp + 1) * 128)
                nc.tensor.matmul(psw_t[:, hp, 0:C],
                                 lhsT=ctx_sb[:, hsl], rhs=wo_sb[:, hp, :],
                                 start=True, stop=True)
            for hp in range(4):
                nc.vector.tensor_scalar_mul(out=weff_sb[:, hp, :],
                                            in0=psw_t[:, hp, 0:C],
                                            scalar1=rs[:, hp:hp + 1])

            # W_comb^T[c,o] = sum_d w_q[d,c] * W_eff^T[d,o]
            wc_ps = psw.tile([128, 2, 512], F32)
            for c2 in range(2):
                csl = slice(c2 * 128, (c2 + 1) * 128)
                for d4 in range(4):
                    nc.tensor.matmul(wc_ps[:, c2, 0:C],
                                     lhsT=wq_sb[:, d4, csl],
                                     rhs=weff_sb[:, d4, :],
                                     start=(d4 == 0), stop=(d4 == 3))
            for c2 in range(2):
                nc.vector.tensor_copy(out=wcombT_sb[:, c2, :], in_=wc_ps[:, c2, 0:C])

        if debug:
            weff32 = small.tile([128, 4, C], F32)
            nc.vector.tensor_copy(out=weff32, in_=weff_sb)
            nc.sync.dma_start(out=dbg["d_weff"], in_=weff32)

        acc_ctx.close()  # release ctx/S PSUM banks before pass B

        # ---------------- PASS B: out = W_comb @ img + b ----------------
        with ExitStack() as bctx:
            pb = bctx.enter_context(tc.tile_pool(name="pb", bufs=pb_bufs))
            pso = bctx.enter_context(tc.tile_pool(name="pso", bufs=pso_bufs, space="PSUM"))
            for i in range(ntb):
                sl = slice(i * tb, (i + 1) * tb)
                out_ps = pso.tile([128, 2, tb], F32)
                for o2 in range(2):
                    osl = slice(o2 * 128, (o2 + 1) * 128)
                    for c2 in range(2):
                        nc.tensor.matmul(out_ps[:, o2, :],
                                         lhsT=wcombT_sb[:, c2, osl],
                                         rhs=img_sb[:, c2, sl],
                                         start=(c2 == 0), stop=(c2 == 1))
                out_sb = pb.tile([128, 2, tb], F32, tag="o")
                for o2 in range(2):
                    nc.scalar.activation(out=out_sb[:, o2, :], in_=out_ps[:, o2, :],
                                         func=AF.Identity,
                                         bias=b_sb[:, o2:o2 + 1])
                nc.sync.dma_start(out=out_d[:, :, sl], in_=out_sb)

    with tile.TileContext(nc) as tc:
        for _rep in range(repeat):
            with ExitStack() as ctx:
                emit(tc, ctx)
        if marker_d is not None:
            with tc.tile_pool(name="mk", bufs=1) as mk:
                m = mk.tile([1, 4], F32)
                nc.vector.memset(m, 1.0)
                nc.sync.dma_start(out=marker_d, in_=m)

    nc.compile()
    return nc


def _prep_inputs(img, w_qkv, w_out, b_out, n_tok=N_TOK):
    imgs = np.ascontiguousarray(img.reshape(B, C, n_tok), dtype=np.float32)
    w_qkv = np.asarray(w_qkv, dtype=np.float32)
    w_kvT = np.ascontiguousarray(w_qkv[DH:3 * DH].T)      # [256, 1024]
    w_q = np.ascontiguousarray(w_qkv[0:DH])               # [512, 256]
    w_outT = np.ascontiguousarray(np.asarray(w_out, dtype=np.float32).T)  # [512, 256]
    b = np.ascontiguousarray(np.asarray(b_out, dtype=np.float32))
    return [
        {"img": imgs[i], "w_kvT": w_kvT, "w_q": w_q, "w_outT": w_outT, "b_out": b}
        for i in range(N_CORES)
    ]


class _Exec:
    """Compile once, execute many times on the 8 cores via PJRT/shard_map."""

    def __init__(self, nc):
        import jax
        import concourse.mybir as mybir
        from jax.experimental.shard_map import shard_map
        from jax.sharding import Mesh, PartitionSpec, NamedSharding
        from concourse.bass2jax import _bass_exec_p, install_neuronx_cc_hook, partition_id_tensor

        install_neuronx_cc_hook()
        self.jax = jax
        in_names, out_names, out_avals = [], [], []
        partition_name = nc.partition_id_tensor.name if nc.partition_id_tensor else None
        for alloc in nc.m.functions[0].allocations:
            if not isinstance(alloc, mybir.MemoryLocationSet):
                continue
            name = alloc.memorylocations[0].name
            if alloc.kind == "ExternalInput":
                if name != partition_name:
                    in_names.append(name)
            elif alloc.kind == "ExternalOutput":
                out_names.append(name)
                out_avals.append(jax.core.ShapedArray(
                    tuple(alloc.tensor_shape), mybir.dt.np(alloc.dtype)))
        self.in_names, self.out_names, self.out_avals = in_names, out_names, out_avals
        n_params = len(in_names)
        all_in_names = in_names + out_names
        if partition_name is not None:
            all_in_names.append(partition_name)

        def _body(*args):
            operands = list(args)
            if partition_name is not None:
                operands.append(partition_id_tensor())
            return tuple(_bass_exec_p.bind(
                *operands,
                out_avals=tuple(out_avals),
                in_names=tuple(all_in_names),
                out_names=tuple(out_names),
                lowering_input_output_aliases=(),
                sim_require_finite=True,
                sim_require_nnan=True,
                nc=nc,
            ))

        devices = jax.devices()[:N_CORES]
        mesh = Mesh(np.asarray(devices), ("core",))
        self._body = _body
        self.mesh = mesh
        self.sharding = NamedSharding(mesh, PartitionSpec("core"))
        n_ops = n_params + len(out_names)
        self.fn = jax.jit(
            shard_map(_body, mesh=mesh,
                      in_specs=(PartitionSpec("core"),) * n_ops,
                      out_specs=(PartitionSpec("core"),) * len(out_names),
                      check_rep=False),
            keep_unused=True,
        )
        self.dev_zeros = [
            jax.device_put(np.zeros((N_CORES * a.shape[0], *a.shape[1:]), a.dtype),
                           self.sharding)
            for a in out_avals
        ]

    def stage(self, in_maps):
        concat = [
            np.concatenate([np.asarray(m[name]) for m in in_maps], axis=0)
            for name in self.in_names
        ]
        return [self.jax.device_put(a, self.sharding) for a in concat]

    def run(self, staged):
        outs = self.fn(*staged, *self.dev_zeros)
        self.jax.block_until_ready(outs)
        return outs

    def results(self, outs):
        per_core = []
        for c in range(N_CORES):
            per_core.append({
                name: np.asarray(outs[i]).reshape(N_CORES, *self.out_avals[i].shape)[c]
                for i, name in enumerate(self.out_names)
            })
        return per_core


_CACHE = {}


def _get_exec():
    if "exec" not in _CACHE:
        _CACHE["exec"] = _Exec(_build_program())
    return _CACHE["exec"]


def kernel(img, w_qkv, w_out, b_out):
    ex = _get_exec()
    staged = ex.stage(_prep_inputs(img, w_qkv, w_out, b_out))
    res = ex.results(ex.run(staged))
    out = np.stack([res[i]["out"] for i in range(N_CORES)])
    return out.reshape(B, C, X, Y)
